# revision 7
# baseline (speedup 1.0000x reference)
"""BranchRoute v2: indirect-scatter stores that skip unrouted rows.

Math per 128-token tile: z_j = sum_d x*w_j (DVE tensor_tensor_reduce),
m_j = z_j > -b_j, row index idx_j = m_j ? (128*t + p) : 2048 (OOB).
Outputs are written with gpsimd indirect scatter DMAs with
bounds_check=1023, oob_is_err=False: unrouted rows are silently
skipped, so HBM write traffic drops from 48 MiB/core (dense) to
~28 MiB/core expected (P(route)=1/2 per branch).  The output DRAM
buffers are donated zero-filled by the runner, so skipped rows are
exactly the zeros the reference produces.

Engines:
  sync (SP/HWDGE): weight/bias broadcast + x tile loads (2 MiB each).
  DVE: per tile, two fused multiply+reduce ops for the gate, the mask/
    index arithmetic (tiny [P,1] ops), and oc = x * (m0+m1).
  gpsimd (SWDGE): iota setup + the three indirect scatters per tile
    (x0 and x1 scatter straight from the x tile; combined from oc).
  ACT: idle.

Expected HBM traffic/core: 16 MiB read + ~28 MiB write = 44 MiB
-> ~128 us at 358 GB/s (vs 187 us dense roofline, 214 us baseline).
"""

import sys

import numpy as np

sys.path.insert(0, "/opt/trn_rl_repo")

import concourse.bass as bass
from concourse import mybir
from concourse.bass_utils import run_bass_kernel_spmd

N_CORES = 8
N, D = 8192, 4096
SHARD = N // N_CORES  # 1024 tokens per core
P = 128
NT = SHARD // P  # 8 tiles per core
BIG = 2048  # OOB row sentinel (> SHARD-1 bounds check -> row skipped)
F32 = mybir.dt.float32
I32 = mybir.dt.int32
Alu = mybir.AluOpType

_CACHE = {}


def _build(nt=NT, n_pass=1):
    nc = bass.Bass()
    x_in = nc.dram_tensor("x", [SHARD, D], F32, kind="ExternalInput")
    gw_in = nc.dram_tensor("gate_w", [D, 2], F32, kind="ExternalInput")
    gb_in = nc.dram_tensor("gate_b", [2], F32, kind="ExternalInput")
    x0_out = nc.dram_tensor("x0", [SHARD, D], F32, kind="ExternalOutput")
    x1_out = nc.dram_tensor("x1", [SHARD, D], F32, kind="ExternalOutput")
    xc_out = nc.dram_tensor("combined", [SHARD, D], F32, kind="ExternalOutput")

    NPT = nt * n_pass  # total tile iterations (n_pass > 1: timing loops)

    def tid(it):
        return it % nt

    from contextlib import ExitStack

    with ExitStack() as ctx:
        sb = lambda name, *shape: ctx.enter_context(
            nc.sbuf_tensor(name, list(shape), F32)
        )
        sbi = lambda name, *shape: ctx.enter_context(
            nc.sbuf_tensor(name, list(shape), I32)
        )
        sem = lambda name: ctx.enter_context(nc.semaphore(name))
        gwb = sb("gwb", P, 2 * D)  # interleaved w0/w1 bcast
        w0 = sb("w0", P, D)  # de-interleaved contiguous copies
        w1 = sb("w1", P, D)
        bb = sb("bb", P, 2)  # bias bcast
        nb = sb("nb", P, 2)  # -bias
        xt = [sb(f"xt{i}", P, D) for i in range(3)]
        oc = [sb(f"oc{i}", P, D) for i in range(3)]
        prod = ctx.enter_context(nc.psum_tensor("prod", [P, D], F32))
        z = sb("z", P, 2)
        m = sb("m", P, 2)
        ms = sb("ms", P, 1)
        mc = sb("mc", P, 1)
        pmt_i = sbi("pmt_i", P, nt)  # iota: p + 128*t - BIG
        pmt = sb("pmt", P, nt)  # f32 copy for DVE select math
        idx0 = [sbi(f"idx0_{i}", P, 1) for i in range(3)]
        idx1 = [sbi(f"idx1_{i}", P, 1) for i in range(3)]
        idxc = [sbi(f"idxc_{i}", P, 1) for i in range(3)]
        setup_sem = sem("setup_sem")
        giota = sem("giota")
        inx = [sem(f"inx{i}") for i in range(3)]
        sx0 = [sem(f"sx0{i}") for i in range(3)]
        sx1 = [sem(f"sx1{i}") for i in range(3)]
        sxc = [sem(f"sxc{i}") for i in range(3)]
        vec_sem = sem("vec_sem")
        block = ctx.enter_context(nc.Block())
        # de-interleaved strided views of the broadcast weights [P, D]
        gw_v = gwb[:].rearrange("p (d t) -> p t d", t=2)
        w0v = gw_v[:, 0:1, :].rearrange("p one d -> p (one d)")
        w1v = gw_v[:, 1:2, :].rearrange("p one d -> p (one d)")

        # vec_sem: 4 setup ops, then 10 ops per tile
        V = lambda it, k: 4 + 10 * it + k

        def x_done(it):  # x-load completions for slot it%3 up to tile it
            return 16 * (it // 3 + 1)

        def sc_done(it):  # scatter completions on slot sem it%3 up to tile it
            return 16 * (it // 3 + 1)

        @block.sync
        def _(sync):
            gw_flat = gw_in[:, :].rearrange("d t -> (d t)")
            sync.dma_start(
                gwb[:],
                bass.AP(gw_flat.tensor, gw_flat.offset, [[0, P], [1, 2 * D]]),
            ).then_inc(setup_sem, 16)
            gb_flat = gb_in[:]
            sync.dma_start(
                bb[:], bass.AP(gb_flat.tensor, gb_flat.offset, [[0, P], [1, 2]])
            ).then_inc(setup_sem, 16)
            for it in range(min(3, NPT)):
                r = bass.ts(tid(it), P)
                sync.dma_start(xt[it][:], x_in[r, :]).then_inc(inx[it], 16)
            for it in range(NPT):
                if it + 3 < NPT:
                    # xt slot free once tile it's readers are done:
                    # scatters x0/x1 (DMA) and DVE ops (products + oc).
                    sync.wait_ge(sx0[it % 3], sc_done(it))
                    sync.wait_ge(sx1[it % 3], sc_done(it))
                    sync.wait_ge(vec_sem, V(it, 10))
                    rn = bass.ts(tid(it + 3), P)
                    s = (it + 3) % 3
                    sync.dma_start(xt[s][:], x_in[rn, :]).then_inc(inx[s], 16)
            for sem_trip in (sx0, sx1, sxc):
                for j in range(3):
                    n_j = (NPT - j + 2) // 3  # tiles with it%3 == j
                    if n_j > 0:
                        sync.wait_ge(sem_trip[j], 16 * n_j)

        @block.vector
        def _(vector):
            vector.wait_ge(setup_sem, 32)
            nc.vector.tensor_scalar_mul(nb[:], bb[:], -1.0).then_inc(vec_sem, 1)
            vector.wait_ge(giota, 1)
            nc.vector.tensor_copy(pmt[:], pmt_i[:]).then_inc(vec_sem, 1)
            nc.vector.tensor_copy(w0[:], w0v).then_inc(vec_sem, 1)
            nc.vector.tensor_copy(w1[:], w1v).then_inc(vec_sem, 1)
            for it in range(NPT):
                s = it % 3
                t = tid(it)
                vector.wait_ge(inx[s], x_done(it))
                if it == 0:
                    vector.wait_ge(vec_sem, 4)  # setup ops drained
                nc.vector.scalar_tensor_tensor(
                    out=prod[:],
                    in0=xt[s][:],
                    scalar=1.0,
                    in1=w0[:],
                    op0=Alu.mult,
                    op1=Alu.mult,
                    accum_out=z[:, 0:1],
                ).then_inc(vec_sem, 1)
                vector.wait_ge(vec_sem, V(it, 1))  # prod WAW
                nc.vector.scalar_tensor_tensor(
                    out=prod[:],
                    in0=xt[s][:],
                    scalar=1.0,
                    in1=w1[:],
                    op0=Alu.mult,
                    op1=Alu.mult,
                    accum_out=z[:, 1:2],
                ).then_inc(vec_sem, 1)
                vector.wait_ge(vec_sem, V(it, 2))  # z writes drained
                nc.vector.tensor_scalar(
                    out=m[:, 0:1],
                    in0=z[:, 0:1],
                    scalar1=nb[:, 0:1],
                    scalar2=None,
                    op0=Alu.is_gt,
                ).then_inc(vec_sem, 1)
                nc.vector.tensor_scalar(
                    out=m[:, 1:2],
                    in0=z[:, 1:2],
                    scalar1=nb[:, 1:2],
                    scalar2=None,
                    op0=Alu.is_gt,
                ).then_inc(vec_sem, 1)
                vector.wait_ge(vec_sem, V(it, 4))  # m writes drained
                nc.vector.tensor_scalar(
                    out=idx0[s][:],
                    in0=m[:, 0:1],
                    scalar1=pmt[:, t : t + 1],
                    scalar2=float(BIG),
                    op0=Alu.mult,
                    op1=Alu.add,
                ).then_inc(vec_sem, 1)
                nc.vector.tensor_scalar(
                    out=idx1[s][:],
                    in0=m[:, 1:2],
                    scalar1=pmt[:, t : t + 1],
                    scalar2=float(BIG),
                    op0=Alu.mult,
                    op1=Alu.add,
                ).then_inc(vec_sem, 1)
                nc.vector.tensor_add(ms[:], m[:, 0:1], m[:, 1:2]).then_inc(
                    vec_sem, 1
                )
                vector.wait_ge(vec_sem, V(it, 7))  # ms drained
                nc.vector.tensor_scalar(
                    out=mc[:],
                    in0=ms[:],
                    scalar1=0.5,
                    scalar2=None,
                    op0=Alu.is_gt,
                ).then_inc(vec_sem, 1)
                vector.wait_ge(vec_sem, V(it, 8))  # mc drained
                if it >= 3:
                    # xc(it-3) done: frees both idxc[it%3] and oc[it%3]
                    vector.wait_ge(sxc[it % 3], sc_done(it - 3))
                nc.vector.tensor_scalar(
                    out=idxc[s][:],
                    in0=mc[:],
                    scalar1=pmt[:, t : t + 1],
                    scalar2=float(BIG),
                    op0=Alu.mult,
                    op1=Alu.add,
                ).then_inc(vec_sem, 1)
                nc.vector.tensor_scalar_mul(
                    oc[it % 3][:], xt[s][:], ms[:]
                ).then_inc(vec_sem, 1)

        @block.gpsimd
        def _(gpsimd):
            bc_reg = nc.gpsimd.to_reg(SHARD - 1)  # shared bounds-check reg
            nc.gpsimd.iota(
                pmt_i[:],
                pattern=[[P, nt]],
                base=-BIG,
                channel_multiplier=1,
            ).then_inc(giota, 1)
            for it in range(NPT):
                s = it % 3
                b = it % 3
                gpsimd.wait_ge(vec_sem, V(it, 5))
                if it >= 3:
                    gpsimd.wait_ge(sx0[b], sc_done(it - 3))  # sem slot free
                nc.gpsimd.indirect_dma_start(
                    out=x0_out[:, :],
                    out_offset=bass.IndirectOffsetOnAxis(
                        ap=idx0[s][:, 0:1], axis=0
                    ),
                    in_=xt[s][:],
                    in_offset=None,
                    bounds_check=bc_reg,
                    oob_is_err=False,
                ).then_inc(sx0[b], 16)
                gpsimd.wait_ge(vec_sem, V(it, 6))
                if it >= 3:
                    gpsimd.wait_ge(sx1[b], sc_done(it - 3))  # sem slot free
                nc.gpsimd.indirect_dma_start(
                    out=x1_out[:, :],
                    out_offset=bass.IndirectOffsetOnAxis(
                        ap=idx1[s][:, 0:1], axis=0
                    ),
                    in_=xt[s][:],
                    in_offset=None,
                    bounds_check=bc_reg,
                    oob_is_err=False,
                ).then_inc(sx1[b], 16)
                gpsimd.wait_ge(vec_sem, V(it, 10))
                if it >= 3:
                    gpsimd.wait_ge(sxc[b], sc_done(it - 3))  # sem slot free
                nc.gpsimd.indirect_dma_start(
                    out=xc_out[:, :],
                    out_offset=bass.IndirectOffsetOnAxis(
                        ap=idxc[s][:, 0:1], axis=0
                    ),
                    in_=oc[b][:],
                    in_offset=None,
                    bounds_check=bc_reg,
                    oob_is_err=False,
                ).then_inc(sxc[b], 16)

    nc.finalize()
    return nc


def _get_nc(n_pass=1):
    key = ("nc", n_pass)
    if key not in _CACHE:
        _CACHE[key] = _build(n_pass=n_pass)
    return _CACHE[key]


def _get_runner(n_pass=1):
    """Build (once) a jitted 8-core shard_map runner for the bass module,
    mirroring bass2jax.run_bass_via_pjrt but cached across calls."""
    key = ("fn", n_pass)
    if key in _CACHE:
        return _CACHE[key]
    import jax
    from jax.sharding import Mesh, PartitionSpec
    from jax.experimental.shard_map import shard_map
    from concourse import bass2jax

    nc = _get_nc(n_pass)
    bass2jax.install_neuronx_cc_hook()
    partition_name = (
        nc.partition_id_tensor.name if nc.partition_id_tensor else None
    )
    in_names, out_names, out_avals = [], [], []
    for alloc in nc.m.functions[0].allocations:
        if not isinstance(alloc, mybir.MemoryLocationSet):
            continue
        name = alloc.memorylocations[0].name
        if alloc.kind == "ExternalInput":
            if name != partition_name:
                in_names.append(name)
        elif alloc.kind == "ExternalOutput":
            out_names.append(name)
            shape = tuple(alloc.tensor_shape)
            out_avals.append(
                jax.core.ShapedArray(shape, mybir.dt.np(alloc.dtype))
            )
    n_params = len(in_names)
    n_outs = len(out_avals)
    all_names = in_names + out_names
    if partition_name is not None:
        all_names.append(partition_name)
    donate = tuple(range(n_params, n_params + n_outs))

    def _body(*args):
        operands = list(args)
        if partition_name is not None:
            operands.append(bass2jax.partition_id_tensor())
        outs = bass2jax._bass_exec_p.bind(
            *operands,
            out_avals=tuple(out_avals),
            in_names=tuple(all_names),
            out_names=tuple(out_names),
            lowering_input_output_aliases=(),
            sim_require_finite=True,
            sim_require_nnan=True,
            nc=nc,
        )
        return tuple(outs)

    devices = jax.devices()[:N_CORES]
    mesh = Mesh(np.asarray(devices), ("core",))
    fn = jax.jit(
        shard_map(
            _body,
            mesh=mesh,
            in_specs=(PartitionSpec("core"),) * (n_params + n_outs),
            out_specs=(PartitionSpec("core"),) * n_outs,
            check_rep=False,
        ),
        donate_argnums=donate,
        keep_unused=True,
    )
    runner = (fn, in_names, out_names, out_avals)
    _CACHE[key] = runner
    return runner


def _run_fast(x, gate_w, gate_b, n_pass=1):
    """Execute via the cached jitted runner; returns (x0, x1, combined)."""
    fn, in_names, out_names, out_avals = _get_runner(n_pass)
    full = {"x": x, "gate_w": gate_w, "gate_b": gate_b}
    concat_in = []
    for nm in in_names:
        if nm == "x":
            concat_in.append(x)  # already [N, D]; shard_map splits axis 0
        else:
            a = full[nm]
            concat_in.append(np.concatenate([a] * N_CORES, axis=0))
    zeros = [
        np.zeros((N_CORES * av.shape[0], *av.shape[1:]), av.dtype)
        for av in out_avals
    ]
    outs = fn(*concat_in, *zeros)
    by_name = {nm: np.asarray(o) for nm, o in zip(out_names, outs)}
    return by_name["x0"], by_name["x1"], by_name["combined"]


def _run(x, gate_w, gate_b, trace=False, n_pass=1, **kw):
    x = np.ascontiguousarray(np.asarray(x, dtype=np.float32))
    gate_w = np.ascontiguousarray(np.asarray(gate_w, dtype=np.float32))
    gate_b = np.ascontiguousarray(np.asarray(gate_b, dtype=np.float32))
    assert x.shape == (N, D) and gate_w.shape == (D, 2) and gate_b.shape == (2,)

    nc = _get_nc(n_pass)
    in_maps = [
        {
            "x": x[c * SHARD : (c + 1) * SHARD],
            "gate_w": gate_w,
            "gate_b": gate_b,
        }
        for c in range(N_CORES)
    ]
    res = run_bass_kernel_spmd(
        nc, in_maps, core_ids=list(range(N_CORES)), trace=trace, **kw
    )
    x0 = np.concatenate([res.results[c]["x0"] for c in range(N_CORES)], axis=0)
    x1 = np.concatenate([res.results[c]["x1"] for c in range(N_CORES)], axis=0)
    xc = np.concatenate(
        [res.results[c]["combined"] for c in range(N_CORES)], axis=0
    )
    return (x0, x1, xc), res


def kernel(x, gate_w, gate_b):
    x = np.ascontiguousarray(np.asarray(x, dtype=np.float32))
    gate_w = np.ascontiguousarray(np.asarray(gate_w, dtype=np.float32))
    gate_b = np.ascontiguousarray(np.asarray(gate_b, dtype=np.float32))
    assert x.shape == (N, D) and gate_w.shape == (D, 2) and gate_b.shape == (2,)
    x0, x1, xc = _run_fast(x, gate_w, gate_b)
    return (x0, x1, xc)

